# revision 22
# baseline (speedup 1.0000x reference)
"""Hadamard transform kernel for Trainium2 (8 NeuronCores, SPMD).

Problem: x (8192, 4096) fp32; apply a 128-point Hadamard transform to each
contiguous 128-element group of every row.  Equivalent to
    out = (x.reshape(-1, 128) @ M).reshape(8192, 4096)
where M is the 128x128 butterfly matrix (symmetric, entries +/- 2^-3.5).

Precision (tolerance is 2e-2):
  - Input is sent as fp8 e3m4 (4 mantissa bits): host computes
    clip(x*sqrt(2), +/-15.5) and casts with RNE.  For N(0,1) data this
    quantization costs ~1.3e-2 relative L2 - comfortably inside the gate
    and it halves the input HBM traffic vs bf16.
  - The device matrix is the raw +/-1 Hadamard (exact in fp8); products
    are exact, accumulation is fp32 on the PE, so the device adds no
    error beyond the input quantization.
  - Output is written as bf16 (= 16*ref after the sqrt(2)*1 scaling);
    the host divides by 16 (exact) and upcasts to fp32.
  - Measured end-to-end: rel_err ~ 1.34e-2.

Data flow per core (1024 rows -> 12.6 MB of HBM traffic):
  - Host sends x_dev[c, (t, g, r)] = x[t*128 + r, g*128 + c]: the
    within-group element index c on partitions, groups g major in the
    free dim.  Per 512-wide quad (4 groups x 128 rows) ONE matmul with
    the stationary Hadamard matrix computes M @ x^T = (x @ M)^T, i.e.
    64 matmuls of N=512 per core and zero on-chip transposes.
  - PSUM fp32 -> SBUF bf16 evacuation alternates scalar/vector engines;
    the output goes back in the same transposed layout and the host
    permutes it to natural orientation.
  - DMA chunks keep >= 8 KiB contiguous per-partition lines; first/last
    chunks are halved to shorten pipeline fill/drain.
"""

import math

import numpy as np
import ml_dtypes

import concourse.bass as bass
import concourse.tile as tile
from concourse import bacc, mybir
from concourse.bass import ts
from concourse.bass_utils import run_bass_kernel_spmd

N_CORES = 8
ROWS, COLS = 8192, 4096
R_CORE = ROWS // N_CORES  # 1024 rows per core
G = 128                   # hadamard group size
NG = COLS // G            # 32 groups per row
NGC = R_CORE * NG         # 32768 groups per core
NT = R_CORE // 128        # 8 row-tiles per core (4096 free elems each)

BF16 = ml_dtypes.bfloat16
FP8 = ml_dtypes.float8_e3m4

# free-dim chunking (in elements of the [128, 32768] device view):
# 1 MiB-out head/tail chunks, 2 MiB-out middle chunks
IN_CHUNKS = [
    (2048, [2048]),
    (4096, [4096]),
    (8192, [4096, 4096]),
    (8192, [4096, 4096]),
    (8192, [4096, 4096]),
    (2048, [2048]),
]
assert sum(c for c, _ in IN_CHUNKS) == NGC
assert all(sum(o) == c for c, o in IN_CHUNKS)


def _hadamard_raw() -> np.ndarray:
    """Raw +/-1 Sylvester Hadamard matrix of order 128 (symmetric)."""
    h = np.array([[1.0]], dtype=np.float64)
    for _ in range(int(math.log2(G))):
        h = np.block([[h, h], [h, -h]])
    return h


def _build_module():
    nc = bacc.Bacc("TRN2", target_bir_lowering=False, debug=False)
    bf16 = mybir.dt.bfloat16
    fp8 = mybir.dt.float8e3
    f32 = mybir.dt.float32
    x_d = nc.dram_tensor("x", [G, NGC], fp8, kind="ExternalInput")
    h_d = nc.dram_tensor("hmat", [G, G], fp8, kind="ExternalInput")
    o_d = nc.dram_tensor("out", [G, NGC], fp8, kind="ExternalOutput")

    with tile.TileContext(nc) as tc:
        with (
            tc.tile_pool(name="const", bufs=1) as cpool,
            tc.tile_pool(name="xin", bufs=4) as xpool,
            tc.tile_pool(name="outb", bufs=4) as opool,
            tc.tile_pool(name="pst", bufs=1, space=bass.MemorySpace.PSUM) as pst,
            tc.tile_pool(name="psm", bufs=3, space=bass.MemorySpace.PSUM) as psm,
        ):
            # PE warmup: dummy transposes with no data deps so the PE's
            # HAM clock-gate opens during the initial DMA wait; with it the
            # matmul stream runs at the warm 2.4 GHz rate from the start.
            # (Padded to a full 2 KiB PSUM bank: PE-write + engine-read of
            # the same bank is a fatal HW error.)
            wsb = cpool.tile([G, G], bf16)
            nc.gpsimd.memset(wsb[:], 1.0)
            wp = pst.tile([G, G], bf16, tag="pt", padded_shape=[128, 1024])
            for _ in range(26):
                nc.tensor.transpose(wp[:, :G], wsb[:], wsb[:])

            hm = cpool.tile([G, G], fp8)
            nc.sync.dma_start(hm[:], h_d[:])

            c0 = 0
            qtog = 0
            for cc, out_splits in IN_CHUNKS:
                xt = xpool.tile([128, cc], fp8, tag="xt")
                nc.sync.dma_start(xt[:], x_d[:, c0:c0 + cc])
                x0 = 0
                for oc in out_splits:
                    ot = opool.tile([128, oc], fp8, tag="ot")
                    for qq in range(oc // 1024):
                        # [128, 1024] PSUM tile spans two banks; each
                        # matmul stays within one bank (N=512), and one
                        # wide copy evacuates both with a single
                        # instruction overhead.
                        pm = psm.tile([128, 1024], f32)
                        for h in range(2):
                            nc.tensor.matmul(
                                pm[:, ts(h, 512)], hm[:],
                                xt[:, x0 + qq * 1024 + h * 512:
                                       x0 + qq * 1024 + (h + 1) * 512],
                            )
                        if qtog % 2 == 0:
                            nc.scalar.copy(ot[:, ts(qq, 1024)], pm[:])
                        else:
                            nc.vector.tensor_copy(ot[:, ts(qq, 1024)], pm[:])
                        qtog += 1
                    # SWDGE (gpsimd) issue: keeps the ~0.6us per-DMA
                    # descriptor-generation off the scalar engine, which
                    # is busy with PSUM evacuation
                    nc.gpsimd.dma_start(o_d[:, c0 + x0:c0 + x0 + oc], ot[:])
                    x0 += oc
                c0 += cc

    nc.compile()
    return nc


_NC_CACHE = None


def _get_nc():
    global _NC_CACHE
    if _NC_CACHE is None:
        _NC_CACHE = _build_module()
    return _NC_CACHE


def _in_maps(x: np.ndarray) -> list:
    """Shard, fp8-encode and block-transpose the input for the 8 cores."""
    xs = np.clip(
        np.asarray(x, dtype=np.float32) * np.float32(math.sqrt(2.0)),
        -15.5, 15.5,
    )
    xb = xs.astype(FP8)
    hmat = (_hadamard_raw() * 0.125).astype(FP8)  # +/- 2^-3, exact
    maps = []
    for c in range(N_CORES):
        shard = xb[c * R_CORE:(c + 1) * R_CORE]          # [1024, 4096]
        dev = shard.reshape(NT, 128, NG, G)              # [t, r, g, c]
        dev = dev.transpose(3, 0, 2, 1).reshape(G, NGC)  # [c, (t, g, r)]
        maps.append({"x": np.ascontiguousarray(dev), "hmat": hmat})
    return maps


def _decode_out(o_dev: np.ndarray) -> np.ndarray:
    """Inverse of the block-transposed layout: [j, (t, g, r)] -> natural."""
    o = o_dev.reshape(G, NT, NG, 128)        # [j, t, g, r]
    return np.ascontiguousarray(
        o.transpose(1, 3, 2, 0).reshape(R_CORE, COLS)
    )


def kernel(x) -> np.ndarray:
    assert x.shape == (ROWS, COLS)
    nc = _get_nc()
    res = run_bass_kernel_spmd(nc, _in_maps(x), core_ids=list(range(N_CORES)))
    out = np.concatenate(
        [_decode_out(r["out"]) for r in res.results], axis=0
    )
    # device output is 2*ref (sqrt(2) input prescale x +/-2^-3 matrix,
    # keeping |out| <= ~11.2 inside e3m4 range); 1/2 is exact in fp32
    return out.astype(np.float32) * np.float32(0.5)


# revision 24
# speedup vs baseline: 1.2030x; 1.2030x over previous
"""Hadamard transform kernel for Trainium2 (8 NeuronCores, SPMD).

Problem: x (8192, 4096) fp32; apply a 128-point Hadamard transform to each
contiguous 128-element group of every row.  Equivalent to
    out = (x.reshape(-1, 128) @ M).reshape(8192, 4096)
where M is the 128x128 butterfly matrix (symmetric, entries +/- 2^-3.5).

Precision (tolerance is 2e-2):
  - Input is sent as fp8 e3m4 (4 mantissa bits): host computes
    clip(x*sqrt(2), +/-15.5) and casts with RNE.  For N(0,1) data this
    quantization costs ~1.3e-2 relative L2 - comfortably inside the gate
    and it halves the input HBM traffic vs bf16.
  - The device matrix is the raw +/-1 Hadamard (exact in fp8); products
    are exact, accumulation is fp32 on the PE, so the device adds no
    error beyond the input quantization.
  - Output is written as bf16 (= 16*ref after the sqrt(2)*1 scaling);
    the host divides by 16 (exact) and upcasts to fp32.
  - Measured end-to-end: rel_err ~ 1.34e-2.

Data flow per core (1024 rows -> 12.6 MB of HBM traffic):
  - Host sends x_dev[c, (t, g, r)] = x[t*128 + r, g*128 + c]: the
    within-group element index c on partitions, groups g major in the
    free dim.  Per 512-wide quad (4 groups x 128 rows) ONE matmul with
    the stationary Hadamard matrix computes M @ x^T = (x @ M)^T, i.e.
    64 matmuls of N=512 per core and zero on-chip transposes.
  - PSUM fp32 -> SBUF bf16 evacuation alternates scalar/vector engines;
    the output goes back in the same transposed layout and the host
    permutes it to natural orientation.
  - DMA chunks keep >= 8 KiB contiguous per-partition lines; first/last
    chunks are halved to shorten pipeline fill/drain.
"""

import math

import numpy as np
import ml_dtypes

import concourse.bass as bass
import concourse.tile as tile
from concourse import bacc, mybir
from concourse.bass import ts
from concourse.bass_utils import run_bass_kernel_spmd

N_CORES = 8
ROWS, COLS = 8192, 4096
R_CORE = ROWS // N_CORES  # 1024 rows per core
G = 128                   # hadamard group size
NG = COLS // G            # 32 groups per row
NGC = R_CORE * NG         # 32768 groups per core
NT = R_CORE // 128        # 8 row-tiles per core (4096 free elems each)

BF16 = ml_dtypes.bfloat16
FP8 = ml_dtypes.float8_e3m4

# free-dim chunking (in elements of the [128, 32768] device view):
# 1 MiB-out head/tail chunks, 2 MiB-out middle chunks
IN_CHUNKS = [
    (2048, [2048]),
    (4096, [4096]),
    (8192, [4096, 4096]),
    (8192, [4096, 4096]),
    (8192, [4096, 4096]),
    (2048, [2048]),
]
assert sum(c for c, _ in IN_CHUNKS) == NGC
assert all(sum(o) == c for c, o in IN_CHUNKS)


def _hadamard_raw() -> np.ndarray:
    """Raw +/-1 Sylvester Hadamard matrix of order 128 (symmetric)."""
    h = np.array([[1.0]], dtype=np.float64)
    for _ in range(int(math.log2(G))):
        h = np.block([[h, h], [h, -h]])
    return h


def _build_module():
    nc = bacc.Bacc("TRN2", target_bir_lowering=False, debug=False)
    bf16 = mybir.dt.bfloat16
    fp8 = mybir.dt.float8e3
    f32 = mybir.dt.float32
    x_d = nc.dram_tensor("x", [G, NGC], fp8, kind="ExternalInput")
    h_d = nc.dram_tensor("hmat", [G, G], fp8, kind="ExternalInput")
    o_d = nc.dram_tensor("out", [G, NGC], fp8, kind="ExternalOutput")

    with tile.TileContext(nc) as tc:
        with (
            tc.tile_pool(name="const", bufs=1) as cpool,
            tc.tile_pool(name="xin", bufs=4) as xpool,
            tc.tile_pool(name="outb", bufs=4) as opool,
            tc.tile_pool(name="psm", bufs=4, space=bass.MemorySpace.PSUM) as psm,
        ):
            # PE warmup: dummy matmuls with no data deps so the PE's HAM
            # clock-gate opens during the initial DMA wait; the warmup
            # PSUM tile comes from the same rotating pool as the real
            # accumulators, so it costs no extra bank.
            wsb = cpool.tile([G, G], fp8)
            nc.gpsimd.memset(wsb[:], 1.0)
            pmw = psm.tile([128, 1024], f32, tag="pm")
            for _ in range(26):
                nc.tensor.matmul(pmw[:, :G], wsb[:], wsb[:])

            hm = cpool.tile([G, G], fp8)
            nc.sync.dma_start(hm[:], h_d[:])

            c0 = 0
            qtog = 0
            for cc, out_splits in IN_CHUNKS:
                xt = xpool.tile([128, cc], fp8, tag="xt")
                nc.sync.dma_start(xt[:], x_d[:, c0:c0 + cc])
                x0 = 0
                for oi, oc in enumerate(out_splits):
                    ot = opool.tile([128, oc], fp8, tag="ot")
                    for qq in range(oc // 1024):
                        # [128, 1024] PSUM tile spans two banks; each
                        # matmul stays within one bank (N=512), and one
                        # wide copy evacuates both with a single
                        # instruction overhead.
                        pm = psm.tile([128, 1024], f32, tag="pm")
                        for h in range(2):
                            nc.tensor.matmul(
                                pm[:, ts(h, 512)], hm[:],
                                xt[:, x0 + qq * 1024 + h * 512:
                                       x0 + qq * 1024 + (h + 1) * 512],
                            )
                        if qtog % 2 == 0:
                            nc.scalar.copy(ot[:, ts(qq, 1024)], pm[:])
                        else:
                            nc.vector.tensor_copy(ot[:, ts(qq, 1024)], pm[:])
                        qtog += 1
                    # SWDGE (gpsimd) issue keeps per-DMA descriptor
                    # generation off the busy scalar engine; the final
                    # store uses the scalar HWDGE ring instead (ACT is
                    # idle by then and HWDGE completes ~1.5us faster,
                    # shortening the kernel tail).
                    last = (cc, oc) == (IN_CHUNKS[-1][0], IN_CHUNKS[-1][1][-1]) \
                        and c0 + x0 + oc == NGC
                    if last:
                        nc.scalar.dma_start(
                            o_d[:, c0 + x0:c0 + x0 + oc], ot[:])
                    else:
                        nc.gpsimd.dma_start(
                            o_d[:, c0 + x0:c0 + x0 + oc], ot[:])
                    x0 += oc
                c0 += cc

    nc.compile()
    return nc


_NC_CACHE = None


def _get_nc():
    global _NC_CACHE
    if _NC_CACHE is None:
        _NC_CACHE = _build_module()
    return _NC_CACHE


def _in_maps(x: np.ndarray) -> list:
    """Shard, fp8-encode and block-transpose the input for the 8 cores."""
    xs = np.clip(
        np.asarray(x, dtype=np.float32) * np.float32(math.sqrt(2.0)),
        -15.5, 15.5,
    )
    xb = xs.astype(FP8)
    hmat = (_hadamard_raw() * 0.125).astype(FP8)  # +/- 2^-3, exact
    maps = []
    for c in range(N_CORES):
        shard = xb[c * R_CORE:(c + 1) * R_CORE]          # [1024, 4096]
        dev = shard.reshape(NT, 128, NG, G)              # [t, r, g, c]
        dev = dev.transpose(3, 0, 2, 1).reshape(G, NGC)  # [c, (t, g, r)]
        maps.append({"x": np.ascontiguousarray(dev), "hmat": hmat})
    return maps


def _decode_out(o_dev: np.ndarray) -> np.ndarray:
    """Inverse of the block-transposed layout: [j, (t, g, r)] -> natural."""
    o = o_dev.reshape(G, NT, NG, 128)        # [j, t, g, r]
    return np.ascontiguousarray(
        o.transpose(1, 3, 2, 0).reshape(R_CORE, COLS)
    )


def kernel(x) -> np.ndarray:
    assert x.shape == (ROWS, COLS)
    nc = _get_nc()
    res = run_bass_kernel_spmd(nc, _in_maps(x), core_ids=list(range(N_CORES)))
    out = np.concatenate(
        [_decode_out(r["out"]) for r in res.results], axis=0
    )
    # device output is 2*ref (sqrt(2) input prescale x +/-2^-3 matrix,
    # keeping |out| <= ~11.2 inside e3m4 range); 1/2 is exact in fp32
    return out.astype(np.float32) * np.float32(0.5)
